# revision 4
# baseline (speedup 1.0000x reference)
"""BertGCN fused kernel for 8x TRN2 NeuronCores — single-launch with AllGather.

Math (reference):
    X = label_features @ gc_weight                      # [L, H]
    E = relu(edges @ X + gc_bias)                       # [L, H]
    diag = sum(E * clf_weight, axis=1)                  # [L]
    out = bert_cls @ clf_weight.T + diag[None] + clf_bias[None]   # [B, L]

One SPMD launch over 8 cores (label dim L sharded, 1024 labels/core):
  stage 1: X_c = LF_c @ (GCW*16) in fp8 DoubleRow, written to local DRAM fp8,
           then AllGather across the 8 cores -> full X (8 MB fp8, on
           TOPSP/SDMA silicon, overlaps with stage 3 compute).
  stage 3: logits.T[l, b] = W_c @ bert.T (fp16), held pre-bias in SBUF f16.
  stage 2: E_c = relu(edges_c*L @ X + gc_bias*16L); diag_c = rowsum(E_c * W_c/16L);
           per l-block: emit out.T = logits + diag + clf_bias, DMA to HBM.

All heavy operands are laid out host-side as per-partition-contiguous slabs
(layout + dtype cast only, no FLOPs). B, H, L, F = 2048, 1024, 8192, 1024.
"""

import numpy as np
import ml_dtypes

B, H, L, F = 2048, 1024, 8192, 1024
NCORES = 8
LS = L // NCORES  # 1024 labels per core
P = 128
XS = np.float32(16.0)  # gc_weight pre-scale so X lands well inside fp8 range

LAST_RESULTS = []


def build_kernel():
    from concourse import bacc
    import concourse.mybir as mybir
    import concourse.tile as tile

    dt = mybir.dt
    f32, bf16, f16 = dt.float32, dt.bfloat16, dt.float16
    fp8 = dt.float8e4
    DR = mybir.MatmulPerfMode.DoubleRow
    add = mybir.AluOpType.add
    amax = mybir.AluOpType.max
    mult = mybir.AluOpType.mult

    nc = bacc.Bacc(None, target_bir_lowering=False, debug=False, num_devices=NCORES)

    # stage-1 operands (fp8 DoubleRow layout)
    lf = nc.declare_dram_parameter("lf_dr", [8, P, F], fp8, isOutput=False)
    gcw = nc.declare_dram_parameter("gcw_dr", [P, 4, 2, H], fp8, isOutput=False)
    # stage-2 operands
    gcb = nc.declare_dram_parameter("gcb_row", [1, H], bf16, isOutput=False)
    edg = nc.declare_dram_parameter("edges_slabs", [8, P, L], fp8, isOutput=False)
    cw = nc.declare_dram_parameter("clfw", [LS, H], bf16, isOutput=False)
    cb = nc.declare_dram_parameter("clfb_col", [LS, 1], f32, isOutput=False)
    # stage-3 operands
    cwt = nc.declare_dram_parameter("clfwt_slab", [P, 8, 8, P], f16, isOutput=False)
    brt = nc.declare_dram_parameter("bert_t", [H, B], f16, isOutput=False)
    out = nc.declare_dram_parameter("out_t", [LS, B], f32, isOutput=True)

    KL = L // (2 * P)  # 32 stage-2 k-chunk-pairs (DoubleRow)
    NLB = LS // P      # 8  l-blocks of this core's label shard
    NH2 = H // 512     # 2  h-halves
    NB4 = B // 512     # 4  b-quarters (stage-3 N)
    KH = H // P        # 8  stage-3 k-chunks (over H)

    with tile.TileContext(nc) as tc:
        with (
            tc.tile_pool(name="dram", bufs=1, space="DRAM") as dramp,
            tc.tile_pool(name="const", bufs=1) as constp,
            tc.tile_pool(name="lfp", bufs=2) as lfp,
            tc.tile_pool(name="xop", bufs=2) as xop,
            tc.tile_pool(name="xk", bufs=KL) as xpool,
            tc.tile_pool(name="eslab", bufs=3) as esp,
            tc.tile_pool(name="bstream", bufs=2) as bpool,
            tc.tile_pool(name="cwstream", bufs=2) as cwpool,
            tc.tile_pool(name="opool", bufs=4) as opool,
            tc.tile_pool(name="psA", bufs=4, space="PSUM") as psa,
            tc.tile_pool(name="psB", bufs=4, space="PSUM") as psb,
        ):
            # ---- internal DRAM for the X shard + gathered X ----
            xin_d = dramp.tile([LS, H], fp8, tag="xin", name="xin_d")
            xg_d = dramp.tile([L, H], fp8, tag="xg", addr_space="Shared", name="xg_d")

            # ---- resident constants ----
            gcw_sb = constp.tile([P, 4, 2, H], fp8, tag="gcw")
            nc.sync.dma_start(out=gcw_sb[:], in_=gcw[:])
            gcb_sb = constp.tile([1, H], bf16, tag="gcb")
            nc.sync.dma_start(out=gcb_sb[:], in_=gcb[:])
            ones1 = constp.tile([1, P], bf16, tag="ones1")
            nc.vector.memset(ones1[:], 1.0)
            cwt_sb = constp.tile([P, NLB, KH, P], f16, tag="cwt")
            nc.sync.dma_start(out=cwt_sb[:], in_=cwt[:])
            dscratch = constp.tile([P, H], bf16, tag="dscratch")
            dcol = constp.tile([P, 1], f32, tag="dcol")
            logits_sb = constp.tile([P, NLB, B], f16, tag="logits")
            bias_col = [
                constp.tile([P, 1], f32, tag=f"bias{lb}", name=f"bias{lb}")
                for lb in range(NLB)
            ]

            # ---- stage 1: X_c row-shard = LF_c @ (GCW*16), fp8 DoubleRow ----
            for j in range(NLB):
                lfw = lfp.tile([P, 4, 2, P], fp8, tag="lfw", name=f"lfw{j}")
                nc.sync.dma_start(out=lfw[:], in_=lf[j])
                ps = [
                    psa.tile([P, 512], f32, tag="ps", name=f"psx{j}_{h}")
                    for h in range(NH2)
                ]
                for kc in range(4):
                    for h in range(NH2):
                        nc.tensor.matmul(
                            ps[h][:],
                            lfw[:, kc, :, :],
                            gcw_sb[:, kc, :, 512 * h : 512 * (h + 1)],
                            start=(kc == 0),
                            stop=(kc == 3),
                            perf_mode=DR,
                        )
                xo = xop.tile([P, H], fp8, tag="xo", name=f"xo{j}")
                for h in range(NH2):
                    nc.scalar.copy(xo[:, 512 * h : 512 * (h + 1)], ps[h][:])
                nc.scalar.dma_start(out=xin_d[P * j : P * (j + 1), :], in_=xo[:])

            # ---- AllGather X across the 8 cores (TOPSP/SDMA, overlaps PE) ----
            nc.gpsimd.collective_compute(
                "AllGather",
                mybir.AluOpType.bypass,
                replica_groups=[list(range(NCORES))],
                ins=[xin_d[:].opt()],
                outs=[xg_d[:].opt()],
            )
            # stage-2 rhs tiles [ki, ko, h] <- X rows (2j+ko)*128 + ki
            xg_r = xg_d.rearrange("(j ko p) h -> j p ko h", ko=2, p=P)
            x_sb = []
            for j in range(KL):
                x_sb.append(xpool.tile([P, 2, H], fp8, tag="xk", name=f"x{j}"))
            for j in range(KL):
                nc.gpsimd.dma_start(out=x_sb[j][:], in_=xg_r[j])

            # ---- stage 3: logits.T[l, b] = W_c @ bert.T (fp16), pre-bias ----
            brt_r = brt.rearrange("(k p) b -> p k b", p=P)
            bt_tiles = {}
            for bq in range(2):
                bt_tiles[bq] = bpool.tile([P, KH, 512], f16, tag="bt", name=f"bt{bq}")
                nc.sync.dma_start(
                    out=bt_tiles[bq][:], in_=brt_r[:, :, 512 * bq : 512 * (bq + 1)]
                )
            for bq in range(NB4):
                if bq in bt_tiles:
                    bt_sb = bt_tiles[bq]
                else:
                    bt_sb = bpool.tile([P, KH, 512], f16, tag="bt", name=f"bt{bq}")
                    nc.sync.dma_start(
                        out=bt_sb[:], in_=brt_r[:, :, 512 * bq : 512 * (bq + 1)]
                    )
                for lb in range(NLB):
                    ps = psb.tile([P, 512], f32, tag="pso")
                    for k in range(KH):
                        nc.tensor.matmul(
                            ps[:],
                            cwt_sb[:, lb, k, :],
                            bt_sb[:, k, :],
                            start=(k == 0),
                            stop=(k == KH - 1),
                        )
                    nc.scalar.copy(
                        logits_sb[:, lb, 512 * bq : 512 * (bq + 1)], ps[:]
                    )

            # ---- stage 2: E = relu(edges_c @ X + bias); diag; emit output ----
            for lb in range(NLB):
                eslab = esp.tile([P, KL, 2, P], fp8, tag="eslab", name=f"eslab{lb}")
                nc.sync.dma_start(out=eslab[:], in_=edg[lb])
                cw_sb = cwpool.tile([P, H], bf16, tag="cw", name=f"cw{lb}")
                nc.sync.dma_start(out=cw_sb[:], in_=cw[P * lb : P * (lb + 1), :])
                ps = [
                    psa.tile([P, 512], f32, tag="ps", name=f"pse{lb}_{h}")
                    for h in range(NH2)
                ]
                for k in range(KL):
                    for h in range(NH2):
                        nc.tensor.matmul(
                            ps[h][:],
                            eslab[:, k, :, :],
                            x_sb[k][:, :, 512 * h : 512 * (h + 1)],
                            start=(k == 0),
                            stop=False,
                            perf_mode=DR,
                        )
                for h in range(NH2):
                    # + gc_bias via K=1 accumulation row
                    nc.tensor.matmul(
                        ps[h][:],
                        ones1[:],
                        gcb_sb[:, 512 * h : 512 * (h + 1)],
                        start=False,
                        stop=True,
                    )
                    # fused relu(E)*W product straight out of PSUM
                    nc.vector.scalar_tensor_tensor(
                        dscratch[:, 512 * h : 512 * (h + 1)],
                        ps[h][:],
                        0.0,
                        cw_sb[:, 512 * h : 512 * (h + 1)],
                        op0=amax,
                        op1=mult,
                    )
                nc.vector.tensor_reduce(
                    dcol[:], dscratch[:], axis=mybir.AxisListType.X, op=add
                )
                cb_sb = cwpool.tile([P, 1], f32, tag="cb")
                nc.sync.dma_start(out=cb_sb[:], in_=cb[P * lb : P * (lb + 1), :])
                nc.vector.tensor_add(bias_col[lb][:], dcol[:], cb_sb[:])
                # emit this l-block: out = logits + diag + clf_bias
                for bq in range(NB4):
                    o_sb = opool.tile([P, 512], f32, tag="o")
                    nc.scalar.add(
                        o_sb[:],
                        logits_sb[:, lb, 512 * bq : 512 * (bq + 1)],
                        add=bias_col[lb][:],
                    )
                    nc.scalar.dma_start(
                        out=out[P * lb : P * (lb + 1), 512 * bq : 512 * (bq + 1)],
                        in_=o_sb[:],
                    )

    nc.compile()
    return nc


def _prep_inputs(bert_cls, label_features, edges, gc_weight, gc_bias, clf_weight, clf_bias):
    """Host-side shard/layout/cast prep. Layout + dtype only — no math."""
    bf16 = ml_dtypes.bfloat16
    f8 = ml_dtypes.float8_e4m3

    # lf_dr[c][j, ki, kc, ko, l2] = LF[c*1024 + j*128 + l2, kc*256 + ko*128 + ki]
    lf_all = np.ascontiguousarray(
        label_features.reshape(NCORES, 8, P, 4, 2, P)
        .transpose(0, 1, 5, 3, 4, 2)
        .astype(f8)
        .reshape(NCORES, 8, P, F)
    )
    # gcw_dr[ki, kc, ko, h] = (GCW*16)[kc*256 + ko*128 + ki, h]
    gcw_dr = np.ascontiguousarray(
        (gc_weight * XS).reshape(4, 2, P, H).transpose(2, 0, 1, 3).astype(f8)
    )
    gcb_row = np.ascontiguousarray((gc_bias * (L * XS)).reshape(1, H).astype(bf16))
    bert_t = np.ascontiguousarray(bert_cls.T.astype(np.float16))

    shared = dict(gcw_dr=gcw_dr, gcb_row=gcb_row, bert_t=bert_t)
    maps = []
    for c in range(NCORES):
        sl = slice(c * LS, (c + 1) * LS)
        e_c = edges[sl, :]  # [1024, 8192]
        # DoubleRow fp8 slabs: [lb, ki, kc, ko, j] = e_c[lb*128+j, (2kc+ko)*128+ki] * L
        edges_slabs = np.ascontiguousarray(
            (e_c.reshape(8, P, 32, 2, P) * np.float32(L))
            .transpose(0, 4, 2, 3, 1)
            .astype(f8)
            .reshape(8, P, L)
        )
        w_c = clf_weight[sl, :]  # [1024, 1024]
        # clfwt_slab[i, lb, k, j] = w_c[lb*128+j, k*128+i]
        clfwt_slab = np.ascontiguousarray(
            w_c.reshape(8, P, 8, P).transpose(3, 0, 2, 1).astype(np.float16)
        )
        maps.append(
            dict(
                shared,
                lf_dr=lf_all[c],
                edges_slabs=edges_slabs,
                clfwt_slab=clfwt_slab,
                clfw=np.ascontiguousarray((w_c / (np.float32(L) * XS)).astype(bf16)),
                clfb_col=np.ascontiguousarray(
                    clf_bias[sl].reshape(LS, 1).astype(np.float32)
                ),
            )
        )
    return maps


def kernel(**inputs):
    global LAST_RESULTS
    from concourse.bass_utils import run_bass_kernel_spmd

    inputs = {k: np.asarray(v) for k, v in inputs.items()}
    maps = _prep_inputs(**inputs)

    nc = build_kernel()
    res = run_bass_kernel_spmd(nc, maps, core_ids=list(range(NCORES)))
    LAST_RESULTS = [res]
    out_t = np.concatenate([res.results[c]["out_t"] for c in range(NCORES)], axis=0)
    return np.ascontiguousarray(out_t.T)


if __name__ == "__main__":
    rng = np.random.default_rng(0)
    ins = dict(
        bert_cls=rng.standard_normal((B, H), dtype=np.float32),
        label_features=rng.standard_normal((L, F), dtype=np.float32),
        edges=(rng.random((L, L), dtype=np.float32) / L),
        gc_weight=rng.standard_normal((F, H), dtype=np.float32) / np.sqrt(F),
        gc_bias=np.zeros(H, np.float32),
        clf_weight=rng.standard_normal((L, H), dtype=np.float32) / np.sqrt(H),
        clf_bias=np.zeros(L, np.float32),
    )
    got = kernel(**ins)
    X = ins["label_features"] @ ins["gc_weight"]
    E = np.maximum(ins["edges"] @ X + ins["gc_bias"], 0)
    diag = (E * ins["clf_weight"]).sum(1)
    exp = ins["bert_cls"] @ ins["clf_weight"].T + diag[None, :] + ins["clf_bias"][None, :]
    rel = np.linalg.norm(got - exp) / np.linalg.norm(exp)
    print("rel err:", rel)


# revision 7
# speedup vs baseline: 1.0822x; 1.0822x over previous
"""BertGCN fused kernel for 8x TRN2 NeuronCores — single-launch with AllGather.

Math (reference):
    X = label_features @ gc_weight                      # [L, H]
    E = relu(edges @ X + gc_bias)                       # [L, H]
    diag = sum(E * clf_weight, axis=1)                  # [L]
    out = bert_cls @ clf_weight.T + diag[None] + clf_bias[None]   # [B, L]

One SPMD launch over 8 cores (label dim L sharded, 1024 labels/core):
  stage 1: X_c = LF_c @ (GCW*16) in fp8 DoubleRow, written to local DRAM fp8,
           then AllGather across the 8 cores -> full X (8 MB fp8, on
           TOPSP/SDMA silicon, overlaps with stage 3 compute).
  stage 3: logits.T[l, b] = W_c @ bert.T (fp16), held pre-bias in SBUF f16.
  stage 2: E_c = relu(edges_c*L @ X + gc_bias*16L); diag_c = rowsum(E_c * W_c/16L);
           per l-block: emit out.T = logits + diag + clf_bias, DMA to HBM.

All heavy operands are laid out host-side as per-partition-contiguous slabs
(layout + dtype cast only, no FLOPs). B, H, L, F = 2048, 1024, 8192, 1024.
"""

import numpy as np
import ml_dtypes

B, H, L, F = 2048, 1024, 8192, 1024
NCORES = 8
LS = L // NCORES  # 1024 labels per core
P = 128
XS = np.float32(16.0)  # gc_weight pre-scale so X lands well inside fp8 range

LAST_RESULTS = []


def build_kernel():
    from concourse import bacc
    import concourse.mybir as mybir
    import concourse.tile as tile

    dt = mybir.dt
    f32, bf16, f16 = dt.float32, dt.bfloat16, dt.float16
    fp8 = dt.float8e4
    DR = mybir.MatmulPerfMode.DoubleRow
    add = mybir.AluOpType.add
    amax = mybir.AluOpType.max
    mult = mybir.AluOpType.mult

    nc = bacc.Bacc(None, target_bir_lowering=False, debug=False, num_devices=NCORES)

    # stage-1 operands (fp8 DoubleRow layout)
    lf = nc.declare_dram_parameter("lf_dr", [8, P, F], fp8, isOutput=False)
    gcw = nc.declare_dram_parameter("gcw_dr", [P, 4, 2, H], fp8, isOutput=False)
    # stage-2 operands
    gcb = nc.declare_dram_parameter("gcb_row", [1, H], bf16, isOutput=False)
    edg = nc.declare_dram_parameter("edges_slabs", [8, P, L], fp8, isOutput=False)
    cw = nc.declare_dram_parameter("clfw", [LS, H], bf16, isOutput=False)
    cb = nc.declare_dram_parameter("clfb_col", [LS, 1], f32, isOutput=False)
    # stage-3 operands
    cwt = nc.declare_dram_parameter("clfwt_slab", [P, 8, 8, P], f16, isOutput=False)
    brt = nc.declare_dram_parameter("bert_t", [H, B], f16, isOutput=False)
    out = nc.declare_dram_parameter("out_t", [LS, B], f32, isOutput=True)

    KL = L // (2 * P)  # 32 stage-2 k-chunk-pairs (DoubleRow)
    NLB = LS // P      # 8  l-blocks of this core's label shard
    NH2 = H // 512     # 2  h-halves
    NB4 = B // 512     # 4  b-quarters (stage-3 N)
    KH = H // P        # 8  stage-3 k-chunks (over H)

    with tile.TileContext(nc) as tc:
        with (
            tc.tile_pool(name="dram", bufs=1, space="DRAM") as dramp,
            tc.tile_pool(name="const", bufs=1) as constp,
            tc.tile_pool(name="xop", bufs=2) as xop,
            tc.tile_pool(name="xk", bufs=KL) as xpool,
            tc.tile_pool(name="eslab", bufs=3) as esp,
            tc.tile_pool(name="bstream", bufs=2) as bpool,
            tc.tile_pool(name="cwstream", bufs=2) as cwpool,
            tc.tile_pool(name="opool", bufs=4) as opool,
            tc.tile_pool(name="psA", bufs=4, space="PSUM") as psa,
            tc.tile_pool(name="psB", bufs=4, space="PSUM") as psb,
        ):
            # ---- internal DRAM for the X shard + gathered X ----
            xin_d = dramp.tile([LS, H], fp8, tag="xin", name="xin_d")
            xg_d = dramp.tile([L, H], fp8, tag="xg", addr_space="Shared", name="xg_d")

            # ---- stage-1-critical loads first (gates the AllGather doorbell) ----
            gcw_sb = constp.tile([P, 4, 2, H], fp8, tag="gcw")
            lf_sb = constp.tile([P, NLB, 4, 2, P], fp8, tag="lf")
            lf_r = lf.rearrange("j p (kc ko l) -> p j kc ko l", kc=4, ko=2)
            nc.sync.dma_start(out=gcw_sb[:, 0, :, :], in_=gcw[:, 0, :, :])
            nc.sync.dma_start(out=lf_sb[:, 0:4], in_=lf_r[:, 0:4])
            for kc in range(1, 4):
                nc.sync.dma_start(out=gcw_sb[:, kc, :, :], in_=gcw[:, kc, :, :])
            nc.sync.dma_start(out=lf_sb[:, 4:8], in_=lf_r[:, 4:8])

            ones1 = constp.tile([1, P], bf16, tag="ones1")
            nc.vector.memset(ones1[:], 1.0)
            dscratch = constp.tile([P, H], bf16, tag="dscratch")
            dcol = constp.tile([P, 1], f32, tag="dcol")
            logits_sb = constp.tile([P, NLB, B], f16, tag="logits")
            bias_col = [
                constp.tile([P, 1], f32, tag=f"bias{lb}", name=f"bias{lb}")
                for lb in range(NLB)
            ]

            # ---- stage 1: X_c row-shard = LF_c @ (GCW*16), fp8 DoubleRow ----
            for j in range(NLB):
                ps = [
                    psa.tile([P, 512], f32, tag="ps", name=f"psx{j}_{h}")
                    for h in range(NH2)
                ]
                for kc in range(4):
                    for h in range(NH2):
                        nc.tensor.matmul(
                            ps[h][:],
                            lf_sb[:, j, kc, :, :],
                            gcw_sb[:, kc, :, 512 * h : 512 * (h + 1)],
                            start=(kc == 0),
                            stop=(kc == 3),
                            perf_mode=DR,
                        )
                xo = xop.tile([P, H], fp8, tag="xo", name=f"xo{j}")
                for h in range(NH2):
                    nc.scalar.copy(xo[:, 512 * h : 512 * (h + 1)], ps[h][:])
                nc.scalar.dma_start(out=xin_d[P * j : P * (j + 1), :], in_=xo[:])

            # ---- AllGather X across the 8 cores (TOPSP/SDMA, overlaps PE) ----
            nc.gpsimd.collective_compute(
                "AllGather",
                mybir.AluOpType.bypass,
                replica_groups=[list(range(NCORES))],
                ins=[xin_d[:].opt()],
                outs=[xg_d[:].opt()],
            )
            # stage-2 rhs tiles [ki, ko, h] <- X rows (2j+ko)*128 + ki
            xg_r = xg_d.rearrange("(j ko p) h -> j p ko h", ko=2, p=P)
            x_sb = []
            for j in range(KL):
                x_sb.append(xpool.tile([P, 2, H], fp8, tag="xk", name=f"x{j}"))
            for j in range(KL):
                nc.gpsimd.dma_start(out=x_sb[j][:], in_=xg_r[j])

            # ---- stage 3: logits.T[l, b] = W_c @ bert.T (fp16), pre-bias ----
            # cwt + first two bert quarters on the sync ring (after the
            # stage-1-critical loads); bt2/bt3 retrigger from scalar so the
            # sync ring can stream edges slabs uninterrupted.
            cwt_sb = constp.tile([P, NLB, KH, P], f16, tag="cwt")
            nc.sync.dma_start(out=cwt_sb[:], in_=cwt[:])
            gcb_sb = constp.tile([1, H], bf16, tag="gcb")
            brt_r = brt.rearrange("(k p) b -> p k b", p=P)
            bt_tiles = {}
            for bq in range(2):
                bt_tiles[bq] = bpool.tile([P, KH, 512], f16, tag="bt", name=f"bt{bq}")
                nc.sync.dma_start(
                    out=bt_tiles[bq][:], in_=brt_r[:, :, 512 * bq : 512 * (bq + 1)]
                )
            nc.sync.dma_start(out=gcb_sb[:], in_=gcb[:])
            for bq in range(NB4):
                if bq in bt_tiles:
                    bt_sb = bt_tiles[bq]
                else:
                    bt_sb = bpool.tile([P, KH, 512], f16, tag="bt", name=f"bt{bq}")
                    nc.scalar.dma_start(
                        out=bt_sb[:], in_=brt_r[:, :, 512 * bq : 512 * (bq + 1)]
                    )
                for lb in range(NLB):
                    ps = psb.tile([P, 512], f32, tag="pso")
                    for k in range(KH):
                        nc.tensor.matmul(
                            ps[:],
                            cwt_sb[:, lb, k, :],
                            bt_sb[:, k, :],
                            start=(k == 0),
                            stop=(k == KH - 1),
                        )
                    nc.scalar.copy(
                        logits_sb[:, lb, 512 * bq : 512 * (bq + 1)], ps[:]
                    )

            # ---- stage 2: E = relu(edges_c @ X + bias); diag; emit output ----
            for lb in range(NLB):
                eslab = esp.tile([P, KL, 2, P], fp8, tag="eslab", name=f"eslab{lb}")
                nc.sync.dma_start(out=eslab[:], in_=edg[lb])
                cw_sb = cwpool.tile([P, H], bf16, tag="cw", name=f"cw{lb}")
                nc.sync.dma_start(out=cw_sb[:], in_=cw[P * lb : P * (lb + 1), :])
                ps = [
                    psa.tile([P, 512], f32, tag="ps", name=f"pse{lb}_{h}")
                    for h in range(NH2)
                ]
                for k in range(KL):
                    for h in range(NH2):
                        nc.tensor.matmul(
                            ps[h][:],
                            eslab[:, k, :, :],
                            x_sb[k][:, :, 512 * h : 512 * (h + 1)],
                            start=(k == 0),
                            stop=False,
                            perf_mode=DR,
                        )
                for h in range(NH2):
                    # + gc_bias via K=1 accumulation row
                    nc.tensor.matmul(
                        ps[h][:],
                        ones1[:],
                        gcb_sb[:, 512 * h : 512 * (h + 1)],
                        start=False,
                        stop=True,
                    )
                    # fused relu(E)*W product straight out of PSUM
                    nc.vector.scalar_tensor_tensor(
                        dscratch[:, 512 * h : 512 * (h + 1)],
                        ps[h][:],
                        0.0,
                        cw_sb[:, 512 * h : 512 * (h + 1)],
                        op0=amax,
                        op1=mult,
                    )
                nc.vector.tensor_reduce(
                    dcol[:], dscratch[:], axis=mybir.AxisListType.X, op=add
                )
                cb_sb = cwpool.tile([P, 1], f32, tag="cb")
                nc.sync.dma_start(out=cb_sb[:], in_=cb[P * lb : P * (lb + 1), :])
                nc.vector.tensor_add(bias_col[lb][:], dcol[:], cb_sb[:])
                # emit this l-block: out = logits + diag + clf_bias
                for bq in range(NB4):
                    o_sb = opool.tile([P, 512], f32, tag="o")
                    nc.scalar.add(
                        o_sb[:],
                        logits_sb[:, lb, 512 * bq : 512 * (bq + 1)],
                        add=bias_col[lb][:],
                    )
                    nc.scalar.dma_start(
                        out=out[P * lb : P * (lb + 1), 512 * bq : 512 * (bq + 1)],
                        in_=o_sb[:],
                    )

    nc.compile()
    return nc


def _prep_inputs(bert_cls, label_features, edges, gc_weight, gc_bias, clf_weight, clf_bias):
    """Host-side shard/layout/cast prep. Layout + dtype only — no math."""
    bf16 = ml_dtypes.bfloat16
    f8 = ml_dtypes.float8_e4m3

    # lf_dr[c][j, ki, kc, ko, l2] = LF[c*1024 + j*128 + l2, kc*256 + ko*128 + ki]
    lf_all = np.ascontiguousarray(
        label_features.reshape(NCORES, 8, P, 4, 2, P)
        .transpose(0, 1, 5, 3, 4, 2)
        .astype(f8)
        .reshape(NCORES, 8, P, F)
    )
    # gcw_dr[ki, kc, ko, h] = (GCW*16)[kc*256 + ko*128 + ki, h]
    gcw_dr = np.ascontiguousarray(
        (gc_weight * XS).reshape(4, 2, P, H).transpose(2, 0, 1, 3).astype(f8)
    )
    gcb_row = np.ascontiguousarray((gc_bias * (L * XS)).reshape(1, H).astype(bf16))
    bert_t = np.ascontiguousarray(bert_cls.T.astype(np.float16))

    shared = dict(gcw_dr=gcw_dr, gcb_row=gcb_row, bert_t=bert_t)
    maps = []
    for c in range(NCORES):
        sl = slice(c * LS, (c + 1) * LS)
        e_c = edges[sl, :]  # [1024, 8192]
        # DoubleRow fp8 slabs: [lb, ki, kc, ko, j] = e_c[lb*128+j, (2kc+ko)*128+ki] * L
        edges_slabs = np.ascontiguousarray(
            (e_c.reshape(8, P, 32, 2, P) * np.float32(L))
            .transpose(0, 4, 2, 3, 1)
            .astype(f8)
            .reshape(8, P, L)
        )
        w_c = clf_weight[sl, :]  # [1024, 1024]
        # clfwt_slab[i, lb, k, j] = w_c[lb*128+j, k*128+i]
        clfwt_slab = np.ascontiguousarray(
            w_c.reshape(8, P, 8, P).transpose(3, 0, 2, 1).astype(np.float16)
        )
        maps.append(
            dict(
                shared,
                lf_dr=lf_all[c],
                edges_slabs=edges_slabs,
                clfwt_slab=clfwt_slab,
                clfw=np.ascontiguousarray((w_c / (np.float32(L) * XS)).astype(bf16)),
                clfb_col=np.ascontiguousarray(
                    clf_bias[sl].reshape(LS, 1).astype(np.float32)
                ),
            )
        )
    return maps


def kernel(**inputs):
    global LAST_RESULTS
    from concourse.bass_utils import run_bass_kernel_spmd

    inputs = {k: np.asarray(v) for k, v in inputs.items()}
    maps = _prep_inputs(**inputs)

    nc = build_kernel()
    res = run_bass_kernel_spmd(nc, maps, core_ids=list(range(NCORES)))
    LAST_RESULTS = [res]
    out_t = np.concatenate([res.results[c]["out_t"] for c in range(NCORES)], axis=0)
    return np.ascontiguousarray(out_t.T)


if __name__ == "__main__":
    rng = np.random.default_rng(0)
    ins = dict(
        bert_cls=rng.standard_normal((B, H), dtype=np.float32),
        label_features=rng.standard_normal((L, F), dtype=np.float32),
        edges=(rng.random((L, L), dtype=np.float32) / L),
        gc_weight=rng.standard_normal((F, H), dtype=np.float32) / np.sqrt(F),
        gc_bias=np.zeros(H, np.float32),
        clf_weight=rng.standard_normal((L, H), dtype=np.float32) / np.sqrt(H),
        clf_bias=np.zeros(L, np.float32),
    )
    got = kernel(**ins)
    X = ins["label_features"] @ ins["gc_weight"]
    E = np.maximum(ins["edges"] @ X + ins["gc_bias"], 0)
    diag = (E * ins["clf_weight"]).sum(1)
    exp = ins["bert_cls"] @ ins["clf_weight"].T + diag[None, :] + ins["clf_bias"][None, :]
    rel = np.linalg.norm(got - exp) / np.linalg.norm(exp)
    print("rel err:", rel)


# revision 11
# speedup vs baseline: 1.1226x; 1.0373x over previous
"""BertGCN fused kernel for 8x TRN2 NeuronCores — single-launch with AllGather.

Math (reference):
    X = label_features @ gc_weight                      # [L, H]
    E = relu(edges @ X + gc_bias)                       # [L, H]
    diag = sum(E * clf_weight, axis=1)                  # [L]
    out = bert_cls @ clf_weight.T + diag[None] + clf_bias[None]   # [B, L]

One SPMD launch over 8 cores (label dim L sharded, 1024 labels/core):
  stage 1: X_c = LF_c @ (GCW*16) in fp8 DoubleRow, written to local DRAM fp8,
           then AllGather across the 8 cores -> full X (8 MB fp8, on
           TOPSP/SDMA silicon, overlaps with stage 3 compute).
  stage 3: logits.T[l, b] = W_c @ bert.T (fp16), held pre-bias in SBUF f16.
  stage 2: E_c = relu(edges_c*L @ X + gc_bias*16L); diag_c = rowsum(E_c * W_c/16L);
           per l-block: emit out.T = logits + diag + clf_bias, DMA to HBM.

All heavy operands are laid out host-side as per-partition-contiguous slabs
(layout + dtype cast only, no FLOPs). B, H, L, F = 2048, 1024, 8192, 1024.
"""

import numpy as np
import ml_dtypes

B, H, L, F = 2048, 1024, 8192, 1024
NCORES = 8
LS = L // NCORES  # 1024 labels per core
P = 128
XS = np.float32(16.0)  # gc_weight pre-scale so X lands well inside fp8 range

LAST_RESULTS = []


def build_kernel():
    from concourse import bacc
    import concourse.mybir as mybir
    import concourse.tile as tile

    dt = mybir.dt
    f32, bf16, f16 = dt.float32, dt.bfloat16, dt.float16
    fp8 = dt.float8e4
    DR = mybir.MatmulPerfMode.DoubleRow
    add = mybir.AluOpType.add
    amax = mybir.AluOpType.max
    mult = mybir.AluOpType.mult

    nc = bacc.Bacc(None, target_bir_lowering=False, debug=False, num_devices=NCORES)

    # stage-1 operands (fp8 DoubleRow layout)
    lf = nc.declare_dram_parameter("lf_dr", [8, P, F], fp8, isOutput=False)
    gcw = nc.declare_dram_parameter("gcw_dr", [P, 4, 2, H], fp8, isOutput=False)
    # stage-2 operands
    gcb = nc.declare_dram_parameter("gcb_row", [1, H], bf16, isOutput=False)
    edg = nc.declare_dram_parameter("edges_slabs", [8, P, L], fp8, isOutput=False)
    cw = nc.declare_dram_parameter("clfw", [LS, H], bf16, isOutput=False)
    cb = nc.declare_dram_parameter("clfb_col", [LS, 1], f32, isOutput=False)
    # stage-3 operands
    cwt = nc.declare_dram_parameter("clfwt_slab", [P, 8, 8, P], f16, isOutput=False)
    brt = nc.declare_dram_parameter("bert_t", [H, B], f16, isOutput=False)
    out = nc.declare_dram_parameter("out_t", [LS, B], f32, isOutput=True)

    KL = L // (2 * P)  # 32 stage-2 k-chunk-pairs (DoubleRow)
    NLB = LS // P      # 8  l-blocks of this core's label shard
    NH2 = H // 512     # 2  h-halves
    NB4 = B // 512     # 4  b-quarters (stage-3 N)
    KH = H // P        # 8  stage-3 k-chunks (over H)

    with tile.TileContext(nc) as tc:
        with (
            tc.tile_pool(name="dram", bufs=1, space="DRAM") as dramp,
            tc.tile_pool(name="const", bufs=1) as constp,
            tc.tile_pool(name="xop", bufs=2) as xop,
            tc.tile_pool(name="xk", bufs=KL) as xpool,
            tc.tile_pool(name="eslab", bufs=3) as esp,
            tc.tile_pool(name="bstream", bufs=2) as bpool,
            tc.tile_pool(name="cwstream", bufs=2) as cwpool,
            tc.tile_pool(name="opool", bufs=4) as opool,
            tc.tile_pool(name="psA", bufs=4, space="PSUM") as psa,
            tc.tile_pool(name="psB", bufs=4, space="PSUM") as psb,
        ):
            # ---- internal DRAM for the X shard + gathered X ----
            xin_d = dramp.tile([LS, H], fp8, tag="xin", name="xin_d")
            xg_d = dramp.tile([L, H], fp8, tag="xg", addr_space="Shared", name="xg_d")

            # ---- stage-1-critical loads first (gates the AllGather doorbell) ----
            gcw_sb = constp.tile([P, 4, 2, H], fp8, tag="gcw")
            lf_sb = constp.tile([P, NLB, 4, 2, P], fp8, tag="lf")
            # gcw on the sync ring, lf on the gpsimd ring — parallel arrival
            lf_r = lf.rearrange("j p (kc ko l) -> p j kc ko l", kc=4, ko=2)
            nc.sync.dma_start(out=gcw_sb[:, 0, :, :], in_=gcw[:, 0, :, :])
            nc.gpsimd.dma_start(out=lf_sb[:, 0:4], in_=lf_r[:, 0:4])
            for kc in range(1, 4):
                nc.sync.dma_start(out=gcw_sb[:, kc, :, :], in_=gcw[:, kc, :, :])
            nc.gpsimd.dma_start(out=lf_sb[:, 4:8], in_=lf_r[:, 4:8])

            ones1 = constp.tile([1, P], bf16, tag="ones1")
            nc.vector.memset(ones1[:], 1.0)
            dscratch = constp.tile([P, H], bf16, tag="dscratch")
            dcol = constp.tile([P, 1], f32, tag="dcol")
            logits_sb = constp.tile([P, NLB, B], f16, tag="logits")
            bias_col = [
                constp.tile([P, 1], f32, tag=f"bias{lb}", name=f"bias{lb}")
                for lb in range(NLB)
            ]

            # ---- stage 1: X_c row-shard = LF_c @ (GCW*16), fp8 DoubleRow ----
            for j in range(NLB):
                ps = [
                    psa.tile([P, 512], f32, tag="ps", name=f"psx{j}_{h}")
                    for h in range(NH2)
                ]
                for kc in range(4):
                    for h in range(NH2):
                        nc.tensor.matmul(
                            ps[h][:],
                            lf_sb[:, j, kc, :, :],
                            gcw_sb[:, kc, :, 512 * h : 512 * (h + 1)],
                            start=(kc == 0),
                            stop=(kc == 3),
                            perf_mode=DR,
                        )
                xo = xop.tile([P, H], fp8, tag="xo", name=f"xo{j}")
                for h in range(NH2):
                    nc.scalar.copy(xo[:, 512 * h : 512 * (h + 1)], ps[h][:])
                nc.scalar.dma_start(out=xin_d[P * j : P * (j + 1), :], in_=xo[:])

            # ---- AllGather X across the 8 cores (TOPSP/SDMA, overlaps PE) ----
            nc.gpsimd.collective_compute(
                "AllGather",
                mybir.AluOpType.bypass,
                replica_groups=[list(range(NCORES))],
                ins=[xin_d[:].opt()],
                outs=[xg_d[:].opt()],
            )
            # stage-2 rhs tiles [ki, ko, h] <- X rows (2j+ko)*128 + ki
            xg_r = xg_d.rearrange("(j ko p) h -> j p ko h", ko=2, p=P)
            x_sb = []
            for j in range(KL):
                x_sb.append(xpool.tile([P, 2, H], fp8, tag="xk", name=f"x{j}"))
            for j in range(0, KL, 2):
                nc.gpsimd.dma_start(out=x_sb[j][:], in_=xg_r[j])

            # ---- stage 3: logits.T[l, b] = W_c @ bert.T (fp16), pre-bias ----
            # cwt + first two bert quarters on the sync ring (after the
            # stage-1-critical loads); bt2/bt3 retrigger from scalar so the
            # sync ring can stream edges slabs uninterrupted.
            cwt_sb = constp.tile([P, NLB, KH, P], f16, tag="cwt")
            nc.sync.dma_start(out=cwt_sb[:], in_=cwt[:])
            gcb_sb = constp.tile([1, H], bf16, tag="gcb")
            brt_r = brt.rearrange("(k p) b -> p k b", p=P)
            bt_tiles = {}
            for bq in range(2):
                bt_tiles[bq] = bpool.tile([P, KH, 512], f16, tag="bt", name=f"bt{bq}")
                nc.sync.dma_start(
                    out=bt_tiles[bq][:], in_=brt_r[:, :, 512 * bq : 512 * (bq + 1)]
                )
            nc.sync.dma_start(out=gcb_sb[:], in_=gcb[:])
            for bq in range(NB4):
                if bq in bt_tiles:
                    bt_sb = bt_tiles[bq]
                else:
                    bt_sb = bpool.tile([P, KH, 512], f16, tag="bt", name=f"bt{bq}")
                    nc.scalar.dma_start(
                        out=bt_sb[:], in_=brt_r[:, :, 512 * bq : 512 * (bq + 1)]
                    )
                for lb in range(NLB):
                    ps = psb.tile([P, 512], f32, tag="pso")
                    for k in range(KH):
                        nc.tensor.matmul(
                            ps[:],
                            cwt_sb[:, lb, k, :],
                            bt_sb[:, k, :],
                            start=(k == 0),
                            stop=(k == KH - 1),
                        )
                    nc.scalar.copy(
                        logits_sb[:, lb, 512 * bq : 512 * (bq + 1)], ps[:]
                    )

            # odd X tiles issue from the scalar ring once stage-3 copies are
            # queued (they gate only stage-2, which starts after stage 3)
            for j in range(1, KL, 2):
                nc.scalar.dma_start(out=x_sb[j][:], in_=xg_r[j])

            # ---- stage 2: E = relu(edges_c @ X + bias); diag; emit output ----
            for lb in range(NLB):
                eslab = esp.tile([P, KL, 2, P], fp8, tag="eslab", name=f"eslab{lb}")
                nc.sync.dma_start(out=eslab[:], in_=edg[lb])
                cw_sb = cwpool.tile([P, H], bf16, tag="cw", name=f"cw{lb}")
                nc.sync.dma_start(out=cw_sb[:], in_=cw[P * lb : P * (lb + 1), :])
                ps = [
                    psa.tile([P, 512], f32, tag="ps", name=f"pse{lb}_{h}")
                    for h in range(NH2)
                ]
                for k in range(KL):
                    for h in range(NH2):
                        nc.tensor.matmul(
                            ps[h][:],
                            eslab[:, k, :, :],
                            x_sb[k][:, :, 512 * h : 512 * (h + 1)],
                            start=(k == 0),
                            stop=False,
                            perf_mode=DR,
                        )
                for h in range(NH2):
                    # + gc_bias via K=1 accumulation row
                    nc.tensor.matmul(
                        ps[h][:],
                        ones1[:],
                        gcb_sb[:, 512 * h : 512 * (h + 1)],
                        start=False,
                        stop=True,
                    )
                    # fused relu(E)*W product straight out of PSUM
                    nc.vector.scalar_tensor_tensor(
                        dscratch[:, 512 * h : 512 * (h + 1)],
                        ps[h][:],
                        0.0,
                        cw_sb[:, 512 * h : 512 * (h + 1)],
                        op0=amax,
                        op1=mult,
                    )
                nc.vector.tensor_reduce(
                    dcol[:], dscratch[:], axis=mybir.AxisListType.X, op=add
                )
                cb_sb = cwpool.tile([P, 1], f32, tag="cb")
                nc.sync.dma_start(out=cb_sb[:], in_=cb[P * lb : P * (lb + 1), :])
                nc.vector.tensor_add(bias_col[lb][:], dcol[:], cb_sb[:])
                # emit this l-block: out = logits + diag + clf_bias
                for bq in range(NB4):
                    o_sb = opool.tile([P, 512], f32, tag="o")
                    nc.scalar.add(
                        o_sb[:],
                        logits_sb[:, lb, 512 * bq : 512 * (bq + 1)],
                        add=bias_col[lb][:],
                    )
                    nc.scalar.dma_start(
                        out=out[P * lb : P * (lb + 1), 512 * bq : 512 * (bq + 1)],
                        in_=o_sb[:],
                    )

    nc.compile()
    return nc


def _prep_inputs(bert_cls, label_features, edges, gc_weight, gc_bias, clf_weight, clf_bias):
    """Host-side shard/layout/cast prep. Layout + dtype only — no math."""
    bf16 = ml_dtypes.bfloat16
    f8 = ml_dtypes.float8_e4m3

    # lf_dr[c][j, ki, kc, ko, l2] = LF[c*1024 + j*128 + l2, kc*256 + ko*128 + ki]
    lf_all = np.ascontiguousarray(
        label_features.reshape(NCORES, 8, P, 4, 2, P)
        .transpose(0, 1, 5, 3, 4, 2)
        .astype(f8)
        .reshape(NCORES, 8, P, F)
    )
    # gcw_dr[ki, kc, ko, h] = (GCW*16)[kc*256 + ko*128 + ki, h]
    gcw_dr = np.ascontiguousarray(
        (gc_weight * XS).reshape(4, 2, P, H).transpose(2, 0, 1, 3).astype(f8)
    )
    gcb_row = np.ascontiguousarray((gc_bias * (L * XS)).reshape(1, H).astype(bf16))
    bert_t = np.ascontiguousarray(bert_cls.T.astype(np.float16))

    shared = dict(gcw_dr=gcw_dr, gcb_row=gcb_row, bert_t=bert_t)
    maps = []
    for c in range(NCORES):
        sl = slice(c * LS, (c + 1) * LS)
        e_c = edges[sl, :]  # [1024, 8192]
        # DoubleRow fp8 slabs: [lb, ki, kc, ko, j] = e_c[lb*128+j, (2kc+ko)*128+ki] * L
        edges_slabs = np.ascontiguousarray(
            (e_c.reshape(8, P, 32, 2, P) * np.float32(L))
            .transpose(0, 4, 2, 3, 1)
            .astype(f8)
            .reshape(8, P, L)
        )
        w_c = clf_weight[sl, :]  # [1024, 1024]
        # clfwt_slab[i, lb, k, j] = w_c[lb*128+j, k*128+i]
        clfwt_slab = np.ascontiguousarray(
            w_c.reshape(8, P, 8, P).transpose(3, 0, 2, 1).astype(np.float16)
        )
        maps.append(
            dict(
                shared,
                lf_dr=lf_all[c],
                edges_slabs=edges_slabs,
                clfwt_slab=clfwt_slab,
                clfw=np.ascontiguousarray((w_c / (np.float32(L) * XS)).astype(bf16)),
                clfb_col=np.ascontiguousarray(
                    clf_bias[sl].reshape(LS, 1).astype(np.float32)
                ),
            )
        )
    return maps


def kernel(**inputs):
    global LAST_RESULTS
    from concourse.bass_utils import run_bass_kernel_spmd

    inputs = {k: np.asarray(v) for k, v in inputs.items()}
    maps = _prep_inputs(**inputs)

    nc = build_kernel()
    res = run_bass_kernel_spmd(nc, maps, core_ids=list(range(NCORES)))
    LAST_RESULTS = [res]
    out_t = np.concatenate([res.results[c]["out_t"] for c in range(NCORES)], axis=0)
    return np.ascontiguousarray(out_t.T)


if __name__ == "__main__":
    rng = np.random.default_rng(0)
    ins = dict(
        bert_cls=rng.standard_normal((B, H), dtype=np.float32),
        label_features=rng.standard_normal((L, F), dtype=np.float32),
        edges=(rng.random((L, L), dtype=np.float32) / L),
        gc_weight=rng.standard_normal((F, H), dtype=np.float32) / np.sqrt(F),
        gc_bias=np.zeros(H, np.float32),
        clf_weight=rng.standard_normal((L, H), dtype=np.float32) / np.sqrt(H),
        clf_bias=np.zeros(L, np.float32),
    )
    got = kernel(**ins)
    X = ins["label_features"] @ ins["gc_weight"]
    E = np.maximum(ins["edges"] @ X + ins["gc_bias"], 0)
    diag = (E * ins["clf_weight"]).sum(1)
    exp = ins["bert_cls"] @ ins["clf_weight"].T + diag[None, :] + ins["clf_bias"][None, :]
    rel = np.linalg.norm(got - exp) / np.linalg.norm(exp)
    print("rel err:", rel)


# revision 13
# speedup vs baseline: 1.2480x; 1.1116x over previous
"""BertGCN fused kernel for 8x TRN2 NeuronCores — single-launch with AllGather.

Math (reference):
    X = label_features @ gc_weight                      # [L, H]
    E = relu(edges @ X + gc_bias)                       # [L, H]
    diag = sum(E * clf_weight, axis=1)                  # [L]
    out = bert_cls @ clf_weight.T + diag[None] + clf_bias[None]   # [B, L]

One SPMD launch over 8 cores (label dim L sharded, 1024 labels/core):
  stage 1: X_c = LF_c @ (GCW*16) in fp8 DoubleRow, written to local DRAM fp8,
           then AllGather across the 8 cores -> full X (8 MB fp8, on
           TOPSP/SDMA silicon, overlaps with stage 3 compute).
  stage 3: logits.T[l, b] = W_c @ bert.T (fp16), held pre-bias in SBUF f16.
  stage 2: E_c = relu(edges_c*L @ X + gc_bias*16L); diag_c = rowsum(E_c * W_c/16L);
           per l-block: emit out.T = logits + diag + clf_bias, DMA to HBM.

All heavy operands are laid out host-side as per-partition-contiguous slabs
(layout + dtype cast only, no FLOPs). B, H, L, F = 2048, 1024, 8192, 1024.
"""

import numpy as np
import ml_dtypes

B, H, L, F = 2048, 1024, 8192, 1024
NCORES = 8
LS = L // NCORES  # 1024 labels per core
P = 128
XS = np.float32(16.0)  # gc_weight pre-scale so X lands well inside fp8 range

LAST_RESULTS = []


def build_kernel():
    from concourse import bacc
    import concourse.mybir as mybir
    import concourse.tile as tile

    dt = mybir.dt
    f32, bf16, f16 = dt.float32, dt.bfloat16, dt.float16
    fp8 = dt.float8e4
    DR = mybir.MatmulPerfMode.DoubleRow
    add = mybir.AluOpType.add
    amax = mybir.AluOpType.max
    mult = mybir.AluOpType.mult

    nc = bacc.Bacc(None, target_bir_lowering=False, debug=False, num_devices=NCORES)

    # stage-1 operands (fp8 DoubleRow layout)
    lf = nc.declare_dram_parameter("lf_dr", [8, P, F], fp8, isOutput=False)
    gcw = nc.declare_dram_parameter("gcw_dr", [P, 4, 2, H], fp8, isOutput=False)
    # stage-2 operands
    gcb = nc.declare_dram_parameter("gcb_row", [1, H], bf16, isOutput=False)
    edg = nc.declare_dram_parameter("edges_slabs", [8, P, L], fp8, isOutput=False)
    cw = nc.declare_dram_parameter("clfw", [LS, H], bf16, isOutput=False)
    cb = nc.declare_dram_parameter("clfb_col", [LS, 1], f32, isOutput=False)
    # stage-3 operands
    cwt = nc.declare_dram_parameter("clfwt_slab", [P, 8, 8, P], f16, isOutput=False)
    brt = nc.declare_dram_parameter("bert_t", [H, B], f16, isOutput=False)
    out = nc.declare_dram_parameter("out_t", [LS, B], f32, isOutput=True)

    KL = L // (2 * P)  # 32 stage-2 k-chunk-pairs (DoubleRow)
    NLB = LS // P      # 8  l-blocks of this core's label shard
    NH2 = H // 512     # 2  h-halves
    NB4 = B // 512     # 4  b-quarters (stage-3 N)
    KH = H // P        # 8  stage-3 k-chunks (over H)

    with tile.TileContext(nc) as tc:
        with (
            tc.tile_pool(name="dram", bufs=1, space="DRAM") as dramp,
            tc.tile_pool(name="const", bufs=1) as constp,
            tc.tile_pool(name="xop", bufs=2) as xop,
            tc.tile_pool(name="xk", bufs=KL) as xpool,
            tc.tile_pool(name="eslab", bufs=3) as esp,
            tc.tile_pool(name="bstream", bufs=2) as bpool,
            tc.tile_pool(name="cwstream", bufs=2) as cwpool,
            tc.tile_pool(name="opool", bufs=4) as opool,
            tc.tile_pool(name="psA", bufs=4, space="PSUM") as psa,
            tc.tile_pool(name="psB", bufs=4, space="PSUM") as psb,
        ):
            # ---- internal DRAM for the X shard + gathered X ----
            xin_d = dramp.tile([LS, H], fp8, tag="xin", name="xin_d")
            xg_d = dramp.tile([L, H], fp8, tag="xg", addr_space="Shared", name="xg_d")

            # ---- stage-1-critical loads first (gates the AllGather doorbell) ----
            gcw_sb = constp.tile([P, 4, 2, H], fp8, tag="gcw")
            lf_sb = constp.tile([P, NLB, 4, 2, P], fp8, tag="lf")
            # gcw on the sync ring, lf per-j on the gpsimd ring — parallel arrival
            lf_r = lf.rearrange("j p (kc ko l) -> p j kc ko l", kc=4, ko=2)
            nc.sync.dma_start(out=gcw_sb[:, 0, :, :], in_=gcw[:, 0, :, :])
            nc.gpsimd.dma_start(out=lf_sb[:, 0:1], in_=lf_r[:, 0:1])
            for kc in range(1, 4):
                nc.sync.dma_start(out=gcw_sb[:, kc, :, :], in_=gcw[:, kc, :, :])
            for j in range(1, NLB):
                nc.gpsimd.dma_start(out=lf_sb[:, j : j + 1], in_=lf_r[:, j : j + 1])

            ones1 = constp.tile([1, P], bf16, tag="ones1")
            nc.vector.memset(ones1[:], 1.0)
            dscratch = constp.tile([P, H], bf16, tag="dscratch")
            dcol = constp.tile([P, 1], f32, tag="dcol")
            logits_sb = constp.tile([P, NLB, B], f16, tag="logits")
            bias_col = [
                constp.tile([P, 1], f32, tag=f"bias{lb}", name=f"bias{lb}")
                for lb in range(NLB)
            ]

            # ---- stage 1: X_c row-shard = LF_c @ (GCW*16), fp8 DoubleRow ----
            for j in range(NLB):
                xo = xop.tile([P, H], fp8, tag="xo", name=f"xo{j}")
                # per-half accumulation: h0 closes after 4 MMs so its copy
                # overlaps h1's MMs — keeps the PSUM ring from pacing stage 1
                for h in range(NH2):
                    psh = psa.tile([P, 512], f32, tag="ps", name=f"psx{j}_{h}")
                    for kc in range(4):
                        nc.tensor.matmul(
                            psh[:],
                            lf_sb[:, j, kc, :, :],
                            gcw_sb[:, kc, :, 512 * h : 512 * (h + 1)],
                            start=(kc == 0),
                            stop=(kc == 3),
                            perf_mode=DR,
                        )
                    nc.scalar.copy(xo[:, 512 * h : 512 * (h + 1)], psh[:])
                nc.scalar.dma_start(out=xin_d[P * j : P * (j + 1), :], in_=xo[:])

            # ---- AllGather X across the 8 cores (TOPSP/SDMA, overlaps PE) ----
            nc.gpsimd.collective_compute(
                "AllGather",
                mybir.AluOpType.bypass,
                replica_groups=[list(range(NCORES))],
                ins=[xin_d[:].opt()],
                outs=[xg_d[:].opt()],
            )
            # stage-2 rhs tiles [ki, ko, h] <- X rows (2j+ko)*128 + ki
            xg_r = xg_d.rearrange("(j ko p) h -> j p ko h", ko=2, p=P)
            x_sb = []
            for j in range(KL):
                x_sb.append(xpool.tile([P, 2, H], fp8, tag="xk", name=f"x{j}"))
            for j in range(0, KL, 2):
                nc.gpsimd.dma_start(out=x_sb[j][:], in_=xg_r[j])

            # ---- stage 3: logits.T[l, b] = W_c @ bert.T (fp16), pre-bias ----
            # cwt + first two bert quarters on the sync ring (after the
            # stage-1-critical loads); bt2/bt3 retrigger from scalar so the
            # sync ring can stream edges slabs uninterrupted.
            cwt_sb = constp.tile([P, NLB, KH, P], f16, tag="cwt")
            nc.sync.dma_start(out=cwt_sb[:], in_=cwt[:])
            gcb_sb = constp.tile([1, H], bf16, tag="gcb")
            brt_r = brt.rearrange("(k p) b -> p k b", p=P)
            bt_tiles = {}
            for bq in range(2):
                bt_tiles[bq] = bpool.tile([P, KH, 512], f16, tag="bt", name=f"bt{bq}")
                nc.sync.dma_start(
                    out=bt_tiles[bq][:], in_=brt_r[:, :, 512 * bq : 512 * (bq + 1)]
                )
            nc.sync.dma_start(out=gcb_sb[:], in_=gcb[:])
            for bq in range(NB4):
                if bq in bt_tiles:
                    bt_sb = bt_tiles[bq]
                else:
                    bt_sb = bpool.tile([P, KH, 512], f16, tag="bt", name=f"bt{bq}")
                    nc.scalar.dma_start(
                        out=bt_sb[:], in_=brt_r[:, :, 512 * bq : 512 * (bq + 1)]
                    )
                for lb in range(NLB):
                    ps = psb.tile([P, 512], f32, tag="pso")
                    for k in range(KH):
                        nc.tensor.matmul(
                            ps[:],
                            cwt_sb[:, lb, k, :],
                            bt_sb[:, k, :],
                            start=(k == 0),
                            stop=(k == KH - 1),
                        )
                    nc.scalar.copy(
                        logits_sb[:, lb, 512 * bq : 512 * (bq + 1)], ps[:]
                    )

            # odd X tiles issue from the scalar ring once stage-3 copies are
            # queued (they gate only stage-2, which starts after stage 3)
            for j in range(1, KL, 2):
                nc.scalar.dma_start(out=x_sb[j][:], in_=xg_r[j])

            # ---- stage 2: E = relu(edges_c @ X + bias); diag; emit output ----
            for lb in range(NLB):
                eslab = esp.tile([P, KL, 2, P], fp8, tag="eslab", name=f"eslab{lb}")
                nc.sync.dma_start(out=eslab[:], in_=edg[lb])
                cw_sb = cwpool.tile([P, H], bf16, tag="cw", name=f"cw{lb}")
                nc.sync.dma_start(out=cw_sb[:], in_=cw[P * lb : P * (lb + 1), :])
                ps = [
                    psa.tile([P, 512], f32, tag="ps", name=f"pse{lb}_{h}")
                    for h in range(NH2)
                ]
                for k in range(KL):
                    for h in range(NH2):
                        nc.tensor.matmul(
                            ps[h][:],
                            eslab[:, k, :, :],
                            x_sb[k][:, :, 512 * h : 512 * (h + 1)],
                            start=(k == 0),
                            stop=False,
                            perf_mode=DR,
                        )
                for h in range(NH2):
                    # + gc_bias via K=1 accumulation row
                    nc.tensor.matmul(
                        ps[h][:],
                        ones1[:],
                        gcb_sb[:, 512 * h : 512 * (h + 1)],
                        start=False,
                        stop=True,
                    )
                    # fused relu(E)*W product straight out of PSUM
                    nc.vector.scalar_tensor_tensor(
                        dscratch[:, 512 * h : 512 * (h + 1)],
                        ps[h][:],
                        0.0,
                        cw_sb[:, 512 * h : 512 * (h + 1)],
                        op0=amax,
                        op1=mult,
                    )
                nc.vector.tensor_reduce(
                    dcol[:], dscratch[:], axis=mybir.AxisListType.X, op=add
                )
                cb_sb = cwpool.tile([P, 1], f32, tag="cb")
                nc.sync.dma_start(out=cb_sb[:], in_=cb[P * lb : P * (lb + 1), :])
                nc.vector.tensor_add(bias_col[lb][:], dcol[:], cb_sb[:])
                # emit this l-block: out = logits + diag + clf_bias
                for bq in range(NB4):
                    o_sb = opool.tile([P, 512], f32, tag="o")
                    nc.scalar.add(
                        o_sb[:],
                        logits_sb[:, lb, 512 * bq : 512 * (bq + 1)],
                        add=bias_col[lb][:],
                    )
                    nc.scalar.dma_start(
                        out=out[P * lb : P * (lb + 1), 512 * bq : 512 * (bq + 1)],
                        in_=o_sb[:],
                    )

    nc.compile()
    return nc


def _prep_inputs(bert_cls, label_features, edges, gc_weight, gc_bias, clf_weight, clf_bias):
    """Host-side shard/layout/cast prep. Layout + dtype only — no math."""
    bf16 = ml_dtypes.bfloat16
    f8 = ml_dtypes.float8_e4m3

    # lf_dr[c][j, ki, kc, ko, l2] = LF[c*1024 + j*128 + l2, kc*256 + ko*128 + ki]
    lf_all = np.ascontiguousarray(
        label_features.reshape(NCORES, 8, P, 4, 2, P)
        .transpose(0, 1, 5, 3, 4, 2)
        .astype(f8)
        .reshape(NCORES, 8, P, F)
    )
    # gcw_dr[ki, kc, ko, h] = (GCW*16)[kc*256 + ko*128 + ki, h]
    gcw_dr = np.ascontiguousarray(
        (gc_weight * XS).reshape(4, 2, P, H).transpose(2, 0, 1, 3).astype(f8)
    )
    gcb_row = np.ascontiguousarray((gc_bias * (L * XS)).reshape(1, H).astype(bf16))
    bert_t = np.ascontiguousarray(bert_cls.T.astype(np.float16))

    shared = dict(gcw_dr=gcw_dr, gcb_row=gcb_row, bert_t=bert_t)
    maps = []
    for c in range(NCORES):
        sl = slice(c * LS, (c + 1) * LS)
        e_c = edges[sl, :]  # [1024, 8192]
        # DoubleRow fp8 slabs: [lb, ki, kc, ko, j] = e_c[lb*128+j, (2kc+ko)*128+ki] * L
        edges_slabs = np.ascontiguousarray(
            (e_c.reshape(8, P, 32, 2, P) * np.float32(L))
            .transpose(0, 4, 2, 3, 1)
            .astype(f8)
            .reshape(8, P, L)
        )
        w_c = clf_weight[sl, :]  # [1024, 1024]
        # clfwt_slab[i, lb, k, j] = w_c[lb*128+j, k*128+i]
        clfwt_slab = np.ascontiguousarray(
            w_c.reshape(8, P, 8, P).transpose(3, 0, 2, 1).astype(np.float16)
        )
        maps.append(
            dict(
                shared,
                lf_dr=lf_all[c],
                edges_slabs=edges_slabs,
                clfwt_slab=clfwt_slab,
                clfw=np.ascontiguousarray((w_c / (np.float32(L) * XS)).astype(bf16)),
                clfb_col=np.ascontiguousarray(
                    clf_bias[sl].reshape(LS, 1).astype(np.float32)
                ),
            )
        )
    return maps


def kernel(**inputs):
    global LAST_RESULTS
    from concourse.bass_utils import run_bass_kernel_spmd

    inputs = {k: np.asarray(v) for k, v in inputs.items()}
    maps = _prep_inputs(**inputs)

    nc = build_kernel()
    res = run_bass_kernel_spmd(nc, maps, core_ids=list(range(NCORES)))
    LAST_RESULTS = [res]
    out_t = np.concatenate([res.results[c]["out_t"] for c in range(NCORES)], axis=0)
    return np.ascontiguousarray(out_t.T)


if __name__ == "__main__":
    rng = np.random.default_rng(0)
    ins = dict(
        bert_cls=rng.standard_normal((B, H), dtype=np.float32),
        label_features=rng.standard_normal((L, F), dtype=np.float32),
        edges=(rng.random((L, L), dtype=np.float32) / L),
        gc_weight=rng.standard_normal((F, H), dtype=np.float32) / np.sqrt(F),
        gc_bias=np.zeros(H, np.float32),
        clf_weight=rng.standard_normal((L, H), dtype=np.float32) / np.sqrt(H),
        clf_bias=np.zeros(L, np.float32),
    )
    got = kernel(**ins)
    X = ins["label_features"] @ ins["gc_weight"]
    E = np.maximum(ins["edges"] @ X + ins["gc_bias"], 0)
    diag = (E * ins["clf_weight"]).sum(1)
    exp = ins["bert_cls"] @ ins["clf_weight"].T + diag[None, :] + ins["clf_bias"][None, :]
    rel = np.linalg.norm(got - exp) / np.linalg.norm(exp)
    print("rel err:", rel)
